# revision 16
# baseline (speedup 1.0000x reference)
"""Trainium2 Bass kernel for nn_Conv1dBlock (LIF spikes -> Conv1d(k=5, same) -> GroupNorm).

Contract: kernel(**inputs) takes FULL inputs (x [4,64,256,512] f32, conv_w
[256,256,5], conv_b/gamma/beta [256]) and returns the FULL [4,64,256,512] f32
output. Internally shards data-parallel over B across 8 NeuronCores.

Per-core algorithm (B_loc = 8):
  - LIF (VectorE, fp32, 3 ops in place on the state tile, u = 2*v scaling):
      u = 0.5*m + x ; s = (u >= 1) -> bf16 ; m = (u < 1) * u
  - Conv1d as 5 shifted matmuls per (ci_tile, co_tile) accumulated in PSUM.
    Single bf16 weights (spikes are exact in bf16; weight rounding gives
    ~1.7e-3 end-to-end rel err vs the 2e-2 gate).
  - GroupNorm without ever adding conv bias to the [128,512] data:
      r = sum_l y, q = sum_l y^2 (ScalarE activation accum_out)
      t1 = r + 512 b ; t2 = q + 2 b r + 512 b^2   (per-channel, tiny)
      group sums via ones-block matmul; mu/var/rsqrt on 4-8 lanes;
      broadcast back via ones matmul; out = y*A + B on ScalarE where
      A = kappa*gamma, B = (b - mu)*A + beta.
    The whole tail is batched over sample PAIRS and both co-tiles so the
    many tiny DVE ops run 4x less often than per (sample, ct).
"""

import numpy as np
import ml_dtypes

T, B_FULL, C, L, K = 4, 64, 256, 512, 5
N_CORES = 8
B_LOC = B_FULL // N_CORES
G = 8            # groups
GPC = C // G     # 32 channels per group
CT = 2           # 128-channel tiles
EPS = 1e-5
NORM_N = GPC * L  # 32*512 elements per group

_COMPILED = {}


def _build_program():
    import concourse.bass as bass
    import concourse.tile as tile
    from concourse import bacc, mybir

    f32 = mybir.dt.float32
    bf16 = mybir.dt.bfloat16
    Alu = mybir.AluOpType
    Act = mybir.ActivationFunctionType

    nc = bacc.Bacc(
        "TRN2",
        target_bir_lowering=False,
        debug=False,
        num_devices=N_CORES,
    )

    x_d = nc.dram_tensor("x", [T, B_LOC, C, L], f32, kind="ExternalInput").ap()
    # [ci, k, ci_t, co_t, co] single bf16
    w_d = nc.dram_tensor("w", [128, K, 2, CT, 128], bf16, kind="ExternalInput").ap()
    # [co, field, m(rep), co_t]; fields: b, gamma, beta, 512b, 2b, 512b^2
    chan_d = nc.dram_tensor("chan", [128, 6, 2, CT], f32, kind="ExternalInput").ap()
    onesg_d = nc.dram_tensor("onesg", [128, 4], bf16, kind="ExternalInput").ap()
    onesb_d = nc.dram_tensor("onesb", [128, 128], bf16, kind="ExternalInput").ap()
    y_d = nc.dram_tensor("y", [T, B_LOC, C, L], f32, kind="ExternalOutput").ap()

    with tile.TileContext(nc) as tc:
        with (
            tc.tile_pool(name="singles", bufs=1) as singles,
            tc.tile_pool(name="xp", bufs=10) as xp,
            tc.tile_pool(name="sp", bufs=6) as sp,
            tc.tile_pool(name="ysb", bufs=12) as ysb,
            tc.tile_pool(name="smallsb", bufs=4) as smallsb,
            tc.tile_pool(name="ypsum", bufs=5, space="PSUM") as ypsum,
            tc.tile_pool(name="warmp", bufs=1, space="PSUM") as warmp,
            tc.tile_pool(name="spsum", bufs=2, space="PSUM") as spsum,
        ):
            # PE p-state warmup first: dummy matmuls on a memset tile (no
            # DMA dependency) keep PE busy from ~0 so real convs start at
            # full clock
            warm_sb = singles.tile([128, 64], bf16)
            nc.vector.memset(warm_sb[:], 0.25)
            warm_ps = warmp.tile([128, L], f32)
            for _ in range(110):
                nc.tensor.matmul(
                    warm_ps[0:64, 0:64], warm_sb[:], warm_sb[:],
                    start=True, stop=True, skip_group_check=True,
                )
            # input tiles first (LIF gates the pipeline), then weights
            early_x = {}
            for b in range(2):
                xt = xp.tile([128, 2, L], f32)
                nc.sync.dma_start(
                    out=xt[:], in_=x_d[0, b].rearrange("(i p) l -> p i l", p=128)
                )
                early_x[(0, b)] = xt
            w_s = singles.tile([128, K, 2, CT, 128], bf16)
            nc.sync.dma_start(out=w_s[:], in_=w_d[:])
            onesg = singles.tile([128, 4], bf16)
            nc.sync.dma_start(out=onesg[:], in_=onesg_d[:])
            onesb = singles.tile([128, 128], bf16)
            nc.sync.dma_start(out=onesb[:], in_=onesb_d[:])
            chan = singles.tile([128, 6, 2, CT], f32)
            nc.sync.dma_start(out=chan[:], in_=chan_d[:])
            eps_t = singles.tile([128, 1], f32)
            nc.vector.memset(eps_t[:], EPS)
            # pre-load the activation table (Sqrt selects sqrt_and_others,
            # which also holds Copy/Identity/Square) off the critical path
            eps_s = singles.tile([128, 1], f32)
            nc.scalar.activation(out=eps_s[0:1], in_=eps_t[0:1], func=Act.Sqrt)

            # persistent LIF membrane state (u = 2v scaling) per local batch;
            # first written at t=0 (no memset needed)
            m_tiles = []
            for b in range(B_LOC):
                mt = singles.tile([128, 2, L], f32, tag=f"m{b}")
                m_tiles.append(mt)

            # tap -> (rhs_lo, rhs_hi, out_lo, out_hi) column ranges
            tap_slices = []
            for k in range(K):
                d = k - 2
                if d >= 0:
                    tap_slices.append((d, L, 0, L - d))
                else:
                    tap_slices.append((0, L + d, -d, L))

            mm_list = []
            for ci_t in range(2):
                for k in range(K):
                    mm_list.append((ci_t, k))
            mm_list.remove((0, 2))
            mm_list.insert(0, (0, 2))
            n_mm = len(mm_list)

            def tail_front(pend):
                """Pair-tail stage 1: per-channel stat corrections (DVE) and
                group-sum matmuls (PE). Emitted before the next pair's LIF so
                the PE gsum never stalls behind LIF work on DVE."""
                tb_pair, small_ps, stats, statsb, y_sbs = pend
                # t1 = r + 512b -> bf16 hi+lo   (stats [128, m, c, s])
                t1f = smallsb.tile([128, 2, CT], f32)
                nc.vector.tensor_add(
                    out=t1f[:], in0=stats[:, :, :, 0], in1=chan[:, 3]
                )
                nc.vector.tensor_copy(out=statsb[:, 0, :, :, 0], in_=t1f[:])
                nc.vector.tensor_sub(
                    out=statsb[:, 1, :, :, 0], in0=t1f[:], in1=statsb[:, 0, :, :, 0]
                )
                # t2 = q + 2b*r + 512b^2 -> bf16 hi+lo
                t2f = smallsb.tile([128, 2, CT], f32)
                nc.vector.tensor_mul(
                    out=t2f[:], in0=stats[:, :, :, 0], in1=chan[:, 4]
                )
                nc.vector.tensor_add(out=t2f[:], in0=t2f[:], in1=stats[:, :, :, 1])
                nc.vector.tensor_add(out=t2f[:], in0=t2f[:], in1=chan[:, 5])
                nc.vector.tensor_copy(out=statsb[:, 0, :, :, 1], in_=t2f[:])
                nc.vector.tensor_sub(
                    out=statsb[:, 1, :, :, 1], in0=t2f[:], in1=statsb[:, 0, :, :, 1]
                )
                # group sums: hi/lo parts as separate moving columns of ONE
                # matmul; DVE folds the halves after. out [4, (h m c s)]
                for h in range(2):
                    nc.tensor.matmul(
                        small_ps[0:4, 0:8], onesg[:], statsb[:, h],
                        start=(h == 0), stop=(h == 1),
                    )

            def tail_mid(pend):
                """Pair-tail stage 2: mu/kappa chain (DVE+Act), bf16 split,
                broadcast matmuls (PE), A/B coefficients (DVE)."""
                tb_pair, small_ps, stats, statsb, y_sbs = pend
                gsum = small_ps[0:4, 0:8].rearrange("p (m c s) -> p m c s", m=2, c=CT)
                mk = smallsb.tile([128, 2, CT, 2], f32)  # [grp, m, ct, (mu,kappa)]
                m2 = smallsb.tile([4, 2, CT], f32)
                vr = smallsb.tile([4, 2, CT], f32)
                mu_v = mk[0:4, :, :, 0]
                nc.vector.tensor_scalar(
                    out=mu_v, in0=gsum[:, :, :, 0], scalar1=1.0 / NORM_N,
                    scalar2=None, op0=Alu.mult,
                )
                nc.vector.tensor_mul(out=m2[:], in0=mu_v, in1=mu_v)
                nc.vector.scalar_tensor_tensor(
                    out=vr[:], in0=gsum[:, :, :, 1], scalar=1.0 / NORM_N, in1=m2[:],
                    op0=Alu.mult, op1=Alu.subtract,
                )
                nc.scalar.activation(
                    out=vr[:], in_=vr[:], func=Act.Sqrt, bias=eps_t[0:4],
                )
                nc.vector.reciprocal(out=mk[0:4, :, :, 1], in_=vr[:])

                # bf16 2-way split of (mu, kappa) for the broadcast matmul
                mkb = smallsb.tile([128, 2, 2, CT, 2], bf16)  # [p, j, m, c, s]
                nc.gpsimd.memset(mkb[:], 0.0)
                nc.vector.tensor_copy(out=mkb[0:4, 0], in_=mk[0:4])
                nc.vector.tensor_sub(
                    out=mkb[0:4, 1], in0=mk[0:4], in1=mkb[0:4, 0]
                )
                # broadcast: split parts as moving columns of ONE matmul;
                # DVE folds the halves. out [128, (j m c s)]
                for j in range(2):
                    nc.tensor.matmul(
                        small_ps[:, 16:24], onesb[:], mkb[:, j],
                        start=(j == 0), stop=(j == 1),
                    )
                bcv = small_ps[:, 16:24].rearrange("p (m c s) -> p m c s", m=2, c=CT)
                # A = kappa * gamma ; B = (b - mu) * A + beta
                ab = smallsb.tile([128, 2, CT, 2], f32)
                tmp = smallsb.tile([128, 2, CT], f32)
                nc.vector.tensor_mul(
                    out=ab[:, :, :, 0], in0=bcv[:, :, :, 1], in1=chan[:, 1]
                )
                nc.vector.tensor_sub(
                    out=tmp[:], in0=chan[:, 0], in1=bcv[:, :, :, 0]
                )
                nc.vector.tensor_mul(out=tmp[:], in0=tmp[:], in1=ab[:, :, :, 0])
                nc.vector.tensor_add(
                    out=ab[:, :, :, 1], in0=tmp[:], in1=chan[:, 2]
                )
                return ab

            def tail_store(pend, ab, last=False):
                """Pair-tail stage 3: out = y*A + B (affine) and store.
                mi=0 on ScalarE, mi=1 on GpSimd to split the load."""
                tb_pair, small_ps, stats, statsb, y_sbs = pend
                for mi in range(2):
                    t, b = tb_pair[mi]
                    for ct in range(CT):
                        y_sb = y_sbs[mi][ct]
                        if mi == 1 or last:
                            nc.vector.tensor_scalar(
                                out=y_sb[:], in0=y_sb[:],
                                scalar1=ab[:, mi, ct, 0:1],
                                scalar2=ab[:, mi, ct, 1:2],
                                op0=Alu.mult, op1=Alu.add,
                            )
                        else:
                            nc.scalar.activation(
                                out=y_sb[:], in_=y_sb[:], func=Act.Identity,
                                bias=ab[:, mi, ct, 1:2], scale=ab[:, mi, ct, 0:1],
                            )
                        eng = nc.sync if (last and mi == 1) else nc.gpsimd
                        eng.dma_start(
                            out=y_d[t, b].rearrange("(i p) l -> p i l", p=128)[:, ct, :],
                            in_=y_sb[:],
                        )

            pending = None
            cur = None
            for t in range(T):
                for b in range(B_LOC):
                    idx = t * B_LOC + b
                    mi = idx % 2
                    if mi == 0:
                        if pending is not None:
                            tail_front(pending)
                        small_ps = spsum.tile([128, 32], f32)
                        stats = smallsb.tile([128, 2, CT, 2], f32)
                        statsb = smallsb.tile([128, 2, 2, CT, 2], bf16)  # [p, h, m, c, s]
                        cur = ([None, None], small_ps, stats, statsb, [None, None])
                    cur[0][mi] = (t, b)

                    xt = early_x.pop((t, b), None)
                    if xt is None:
                        xt = xp.tile([128, 2, L], f32)
                        nc.sync.dma_start(
                            out=xt[:],
                            in_=x_d[t, b].rearrange("(i p) l -> p i l", p=128),
                        )
                    mt = m_tiles[b]
                    st = sp.tile([128, 2, L], bf16)
                    if t == 0:
                        # m uninitialized: u = x exactly
                        nc.vector.tensor_scalar(
                            out=st[:], in0=xt[:], scalar1=1.0, scalar2=None,
                            op0=Alu.is_ge,
                        )
                        nc.vector.scalar_tensor_tensor(
                            out=mt[:], in0=xt[:], scalar=1.0, in1=xt[:],
                            op0=Alu.is_lt, op1=Alu.mult,
                        )
                    else:
                        # LIF step: u = 0.5*m + x ; s = (u>=1) ; m = (u<1)*u
                        nc.vector.scalar_tensor_tensor(
                            out=mt[:], in0=mt[:], scalar=0.5, in1=xt[:],
                            op0=Alu.mult, op1=Alu.add,
                        )
                        nc.vector.tensor_scalar(
                            out=st[:], in0=mt[:], scalar1=1.0, scalar2=None,
                            op0=Alu.is_ge,
                        )
                        if t < T - 1:
                            nc.vector.scalar_tensor_tensor(
                                out=mt[:], in0=mt[:], scalar=1.0, in1=mt[:],
                                op0=Alu.is_lt, op1=Alu.mult,
                            )

                    # conv + stats per co-tile
                    y_sbs = []
                    for ct in range(CT):
                        yp = ypsum.tile([128, L], f32)
                        for i, (ci_t, k) in enumerate(mm_list):
                            rl, rh, ol, oh = tap_slices[k]
                            nc.tensor.matmul(
                                yp[:, ol:oh],
                                w_s[:, k, ci_t, ct, :],
                                st[:, ci_t, rl:rh],
                                start=(i == 0),
                                stop=(i == n_mm - 1),
                                skip_group_check=True,
                            )
                        y_sb = ysb.tile([128, L], f32)
                        # r = sum_l y  (and copy PSUM -> SBUF)
                        nc.scalar.activation(
                            out=y_sb[:], in_=yp[:], func=Act.Copy,
                            accum_out=cur[2][:, mi, ct, 0:1],
                        )
                        # q = sum_l y^2 (squares PSUM in place; last PSUM use)
                        nc.scalar.activation(
                            out=yp[:], in_=yp[:], func=Act.Square,
                            accum_out=cur[2][:, mi, ct, 1:2],
                        )
                        y_sbs.append(y_sb)
                    cur[4][mi] = y_sbs

                    if mi == 0:
                        if pending is not None:
                            ab = tail_mid(pending)
                            tail_store(pending, ab)
                    else:
                        pending = cur
            tail_front(pending)
            ab = tail_mid(pending)
            tail_store(pending, ab, last=True)

    nc.compile()
    return nc


def _prep_host_inputs(x, conv_w, conv_b, gamma, beta):
    x = np.asarray(x, dtype=np.float32)
    conv_w = np.asarray(conv_w, dtype=np.float32)
    conv_b = np.asarray(conv_b, dtype=np.float32)
    gamma = np.asarray(gamma, dtype=np.float32)
    beta = np.asarray(beta, dtype=np.float32)

    # lhsT tiles: [ci, k, ci_t, co_t, co] single bf16
    Wt = conv_w.transpose(1, 0, 2)                      # [ci_g, co_g, k]
    W6 = Wt.reshape(2, 128, CT, 128, K)                 # [ci_t, ci, co_t, co, k]
    whi = W6.astype(ml_dtypes.bfloat16)
    w_host = np.ascontiguousarray(whi.transpose(1, 4, 0, 2, 3))

    b = conv_b
    fields = np.stack(
        [b, gamma, beta, np.float32(L) * b, np.float32(2.0) * b,
         np.float32(L) * b * b]
    )                                                   # [6, 256]
    chan1 = fields.reshape(6, CT, 128).transpose(2, 0, 1)   # [128, 6, ct]
    chan = np.ascontiguousarray(
        np.broadcast_to(chan1[:, :, None, :], (128, 6, 2, CT))
    )

    onesg = np.zeros((128, 4), ml_dtypes.bfloat16)
    for ci in range(128):
        onesg[ci, ci // GPC] = 1.0
    onesb = np.zeros((128, 128), ml_dtypes.bfloat16)
    for co in range(128):
        onesb[co // GPC, co] = 1.0

    shards = []
    for i in range(N_CORES):
        shards.append(
            {
                "x": np.ascontiguousarray(x[:, i * B_LOC : (i + 1) * B_LOC]),
                "w": w_host,
                "chan": chan,
                "onesg": onesg,
                "onesb": onesb,
            }
        )
    return shards


def kernel(x, conv_w, conv_b, gamma, beta, _trace=False):
    from concourse.bass_utils import run_bass_kernel_spmd

    if "nc" not in _COMPILED:
        _COMPILED["nc"] = _build_program()
    nc = _COMPILED["nc"]

    in_maps = _prep_host_inputs(x, conv_w, conv_b, gamma, beta)
    res = run_bass_kernel_spmd(
        nc, in_maps, list(range(N_CORES)), trace=_trace
    )
    out = np.concatenate([r["y"] for r in res.results], axis=1)
    _COMPILED["last_result"] = res
    return out
